# revision 1
# baseline (speedup 1.0000x reference)
"""Transformer encoder layer (LN -> MHA -> residual -> LN -> FFN(erf-GELU) -> residual)
for Trainium2, data-parallel over batch across 8 NeuronCores (one batch element per core).

Per-core layout strategy: activations are kept feature-major ("transposed", [feat, tok])
through the matmul pipeline so that weight matrices in their natural [in, out] layout can
be used directly as the stationary operand. LayerNorm stats and residuals run in natural
[tok, feat] space; PE transposes bridge the two. Large GEMMs run in float32r (full-rate
fp32 with mantissa rounding; requires K=128-aligned, M=128 shapes); attention and the
second FFN matmul run in bf16. Softmax needs no max-subtraction: scores/8 ~ N(0,1), far
from fp32 exp overflow. The softmax denominator comes free from an appended ones-column
on V; 1/denom is broadcast across partitions with a K=1 matmul.

The PE executes its stream in order, so any PE instruction waiting on a slow cross-engine
chain stalls all later matmuls. Hence: (1) per-head softmax normalization is deferred by
two heads so the reciprocal chain never blocks the PE; (2) the PE transposes that follow
a PSUM->SBUF eviction are deferred by one outer iteration; (3) LayerNorm computes all
per-tile stats before the apply+transpose pass; (4) AV matmuls trail the score matmuls
by two k-tiles so the ACT exp latency is hidden.

SBUF/PSUM pools are statically allocated, so tags are reused across phases:
lnT tiles serve LN1 then LN2; qk/va/at tiles serve QKV/attention then h1 (FFN hidden).
PSUM budget (8 banks): "s" [128,1024]x2 = 4 (scores / qkv accum / FFN), "av" 4 x 1-bank
slots (attention accumulators + bcast + transpose bounce).
"""
import numpy as np
from contextlib import ExitStack

import concourse.bass as bass
import concourse.bacc as bacc
import concourse.tile as tile
from concourse import mybir
from concourse.bass_utils import run_bass_kernel_spmd
from concourse.masks import make_identity

N_CORES = 8
T = 1024        # tokens per core (sequence length)
D = 1024        # d_model
H = 16          # heads
DH = 64         # head dim
F = 4096        # FFN hidden
PT = T // 128   # token tiles
PD = D // 128   # feature tiles
PF = F // 128   # FFN hidden tiles
EPS = 1e-6

FP32 = mybir.dt.float32
FP32R = mybir.dt.float32r
BF16 = mybir.dt.bfloat16
AF = mybir.ActivationFunctionType


def _build():
    nc = bacc.Bacc(None)

    x_d = nc.dram_tensor("x", [T, D], FP32, kind="ExternalInput")
    wq_d = nc.dram_tensor("w_q", [D, D], FP32, kind="ExternalInput")
    wk_d = nc.dram_tensor("w_k", [D, D], FP32, kind="ExternalInput")
    wv_d = nc.dram_tensor("w_v", [D, D], FP32, kind="ExternalInput")
    wo_d = nc.dram_tensor("w_o", [D, D], FP32, kind="ExternalInput")
    w1_d = nc.dram_tensor("w1", [D, F], FP32, kind="ExternalInput")
    w2_d = nc.dram_tensor("w2", [F, D], FP32, kind="ExternalInput")
    out_d = nc.dram_tensor("out", [T, D], FP32, kind="ExternalOutput")

    x_r = x_d.rearrange("(t p) d -> p t d", p=128)           # [128, PT, D]
    wq_r = wq_d.rearrange("(k p) m -> p k m", p=128)         # [128, PD, D]
    wk_r = wk_d.rearrange("(k p) m -> p k m", p=128)
    wv_r = wv_d.rearrange("(k p) m -> p k m", p=128)
    wo_r = wo_d.rearrange("(k p) m -> p k m", p=128)
    w1_r = w1_d.rearrange("(k p) m -> p k m", p=128)         # [128, PD, F]
    w2_r = w2_d.rearrange("(k p) m -> p k m", p=128)         # [128, PF, D]
    out_r = out_d.rearrange("(t p) d -> p t d", p=128)

    with tile.TileContext(nc) as tc:
        with ExitStack() as ctx:
            const = ctx.enter_context(tc.tile_pool(name="const", bufs=1))
            res = ctx.enter_context(tc.tile_pool(name="res", bufs=1))
            wpool = ctx.enter_context(tc.tile_pool(name="wpool", bufs=3))
            w2pool = ctx.enter_context(tc.tile_pool(name="w2pool", bufs=3))
            lnp = ctx.enter_context(tc.tile_pool(name="lnp", bufs=3))
            stp = ctx.enter_context(tc.tile_pool(name="stp", bufs=9))
            invp = ctx.enter_context(tc.tile_pool(name="invp", bufs=3))
            epool = ctx.enter_context(tc.tile_pool(name="epool", bufs=6))
            opool = ctx.enter_context(tc.tile_pool(name="opool", bufs=3))
            obpool = ctx.enter_context(tc.tile_pool(name="obpool", bufs=4))
            dramp = ctx.enter_context(tc.tile_pool(name="dramp", bufs=3, space="DRAM"))
            ps_big = ctx.enter_context(tc.tile_pool(name="ps_big", bufs=2, space="PSUM"))
            ps_av = ctx.enter_context(tc.tile_pool(name="ps_av", bufs=4, space="PSUM"))

            ident = const.tile([128, 128], FP32)
            make_identity(nc, ident)
            eps_t = const.tile([128, 1], FP32)
            nc.vector.memset(eps_t[:], EPS)
            ones_r = const.tile([1, DH], FP32R)
            nc.vector.memset(ones_r[:].bitcast(FP32), 1.0)

            # ---- resident tensors (tags reused across phases) ----
            x_t = [res.tile([128, D], FP32, tag=f"x{t}", name=f"x{t}")
                   for t in range(PT)]
            lnT = [res.tile([128, T], FP32R, tag=f"lnT{k}", name=f"lnT{k}")
                   for k in range(PD)]

            def layernorm_and_transpose(dst):
                """Stats for all token tiles first, then apply + transpose."""
                mvs, istds = [], []
                for t in range(PT):
                    stats = stp.tile([128, 2, 6], FP32, tag="bn")
                    for i in range(2):
                        nc.vector.bn_stats(out=stats[:, i, :],
                                           in_=x_t[t][:, 512 * i:512 * (i + 1)])
                    mv = stp.tile([128, 2], FP32, tag=f"mv{t % 4}")
                    nc.vector.bn_aggr(out=mv[:], in_=stats[:])
                    istd = stp.tile([128, 1], FP32, tag=f"istd{t % 4}")
                    # std = sqrt(var_pop * n/(n-1) + eps); istd = 1/std
                    nc.scalar.activation(istd[:], mv[:, 1:2], AF.Sqrt,
                                         bias=eps_t[:], scale=float(D) / (D - 1))
                    nc.vector.reciprocal(istd[:], istd[:])
                    mvs.append(mv)
                    istds.append(istd)
                for t in range(PT):
                    ln_nat = lnp.tile([128, D], FP32, tag="ln_nat")
                    nc.vector.tensor_scalar(
                        out=ln_nat[:], in0=x_t[t][:], scalar1=mvs[t][:, 0:1],
                        scalar2=istds[t][:], op0=mybir.AluOpType.subtract,
                        op1=mybir.AluOpType.mult)
                    for d8 in range(PD):
                        tp = ps_av.tile([128, 128], FP32, tag="av", name="tp")
                        nc.tensor.transpose(tp[:], ln_nat[:, 128 * d8:128 * (d8 + 1)],
                                            ident[:])
                        nc.vector.tensor_copy(dst[d8][:, 128 * t:128 * (t + 1)], tp[:])

            # ================= Phase 0/1: load x, LN1 -> lnT =================
            for t in range(PT):
                nc.sync.dma_start(out=x_t[t][:], in_=x_r[:, t])
            layernorm_and_transpose(lnT)

            # ================= Phase 2: QKV projections =================
            qT = [res.tile([128, T], BF16, tag=f"qk{m}", name=f"qT{m}")
                  for m in range(PD)]
            kT = [res.tile([128, T], BF16, tag=f"qk{8 + m}", name=f"kT{m}")
                  for m in range(PD)]
            v_aug = [res.tile([128, H, DH + 1], BF16, tag=f"va{t}", name=f"va{t}")
                     for t in range(PT)]
            for t in range(PT):
                nc.vector.memset(v_aug[t][:, :, DH:DH + 1], 1.0)

            for w_r, dest in ((wq_r, qT), (wk_r, kT)):
                for m in range(PD):
                    wslab = wpool.tile([128, PD, 128], FP32R, tag="wslab",
                                       name="wslab")
                    nc.sync.dma_start(
                        out=wslab[:],
                        in_=w_r[:, :, 128 * m:128 * (m + 1)].bitcast(FP32R))
                    for n in range(2):
                        ps = ps_big.tile([128, 512], FP32, tag="s", name="qkv")
                        for k in range(PD):
                            nc.tensor.matmul(
                                ps[:], wslab[:, k, :],
                                lnT[k][:, 512 * n:512 * (n + 1)],
                                start=(k == 0), stop=(k == PD - 1))
                        nc.vector.tensor_copy(dest[m][:, 512 * n:512 * (n + 1)], ps[:])

            # V: compute vT like q/k, then PE-transpose blocks into natural v_aug.
            # The transposes chase an ACT eviction, so defer them by one iteration
            # to keep the in-order PE stream dense.
            pending = []
            for m in range(PD):
                wslab = wpool.tile([128, PD, 128], FP32R, tag="wslab", name="wslab")
                nc.sync.dma_start(
                    out=wslab[:],
                    in_=wv_r[:, :, 128 * m:128 * (m + 1)].bitcast(FP32R))
                for n in range(2):
                    ps = ps_big.tile([128, 512], FP32, tag="s", name="vps")
                    for k in range(PD):
                        nc.tensor.matmul(
                            ps[:], wslab[:, k, :], lnT[k][:, 512 * n:512 * (n + 1)],
                            start=(k == 0), stop=(k == PD - 1))
                    vt = opool.tile([128, 512], FP32, tag="oT", name="vt")
                    nc.scalar.copy(vt[:], ps[:])

                    def emit_v_transposes(vt=vt, m=m, n=n):
                        for j in range(4):
                            t = 4 * n + j
                            tp = ps_av.tile([128, 128], FP32, tag="av", name="tp")
                            nc.tensor.transpose(tp[:], vt[:, 128 * j:128 * (j + 1)],
                                                ident[:])
                            nc.vector.tensor_copy(
                                v_aug[t][:, 2 * m:2 * m + 2, 0:DH],
                                tp[:].rearrange("p (a d) -> p a d", d=DH))
                    pending.append(emit_v_transposes)
                    if len(pending) > 1:
                        pending.pop(0)()
            for fn in pending:
                fn()

            # ================= Phase 3: attention =================
            attnT = [res.tile([128, T], BF16, tag=f"at{m}", name=f"at{m}")
                     for m in range(PD)]
            norm_pending = []

            def emit_head(h):
                # Scores run two k-tiles ahead of the AV matmuls so the PE never
                # waits on the ACT exp in its in-order stream.
                ht, po = h // 2, 64 * (h % 2)
                avs = [ps_av.tile([DH + 1, 512], FP32, tag="av", name="av")
                       for _ in range(2)]
                es = []

                def emit_scores(kt):
                    s = ps_big.tile([128, T], FP32, tag="s")
                    for n in range(2):
                        nc.tensor.matmul(
                            s[:, 512 * n:512 * (n + 1)],
                            kT[ht][po:po + DH, 128 * kt:128 * (kt + 1)],
                            qT[ht][po:po + DH, 512 * n:512 * (n + 1)],
                            start=True, stop=True)
                    e = epool.tile([128, T], BF16, tag="e")
                    nc.scalar.activation(e[:], s[:], AF.Exp, scale=0.125)
                    es.append(e)

                def emit_av(kt):
                    e = es[kt]
                    for n in range(2):
                        nc.tensor.matmul(
                            avs[n][:], v_aug[kt][:, h, :],
                            e[:, 512 * n:512 * (n + 1)],
                            start=(kt == 0), stop=(kt == PT - 1))

                for kt in range(PT):
                    emit_scores(kt)
                    if kt >= 2:
                        emit_av(kt - 2)
                emit_av(PT - 2)
                emit_av(PT - 1)

                # unnormalized head output + denominator on DVE (ACT is backed
                # up with exp work; the av slots must free fast for the next head)
                inv = invp.tile([1, T], FP32R, tag="inv", name="inv")
                for n in range(2):
                    nc.vector.tensor_copy(
                        attnT[ht][po:po + DH, 512 * n:512 * (n + 1)],
                        avs[n][0:DH, :])
                    with nc.allow_low_precision(reason="softmax denom recip"):
                        nc.vector.reciprocal(inv[:, 512 * n:512 * (n + 1)],
                                             avs[n][DH:DH + 1, :])

                # broadcast 1/denom across partitions via a DRAM bounce
                # (SBUF source APs cannot have a 0-step partition dim), then
                # normalize in place on the DVE -- no PE involvement at all
                dinv = dramp.tile([1, T], FP32R, tag="dinv", name="dinv")
                nc.sync.dma_start(out=dinv[:], in_=inv[:])
                invb = invp.tile([128, T], FP32R, tag="invb", name="invb")
                src = dinv[0:1, :]
                nc.sync.dma_start(
                    out=invb[:],
                    in_=bass.AP(tensor=src.tensor, offset=src.offset,
                                ap=[[0, 128]] + list(src.ap[1:])))

                def normalize(ht=ht, po=po, invb=invb):
                    for n in range(2):
                        nc.vector.tensor_mul(
                            attnT[ht][po:po + DH, 512 * n:512 * (n + 1)],
                            attnT[ht][po:po + DH, 512 * n:512 * (n + 1)],
                            invb[po:po + DH, 512 * n:512 * (n + 1)])
                norm_pending.append(normalize)
                if len(norm_pending) > 2:
                    norm_pending.pop(0)()

            wobs = {}

            def load_wob(m):
                wslab = wpool.tile([128, PD, 128], FP32, tag="wslab", name="wslab")
                nc.sync.dma_start(
                    out=wslab[:], in_=wo_r[:, :, 128 * m:128 * (m + 1)])
                wob = w2pool.tile([128, PD, 128], BF16, tag="w2b", name="wob")
                nc.gpsimd.tensor_copy(wob[:], wslab[:])
                wobs[m] = wob

            for h in range(H):
                emit_head(h)
                if h >= H - 3 and len(wobs) < 3:   # warm up O weights on gpsimd
                    load_wob(len(wobs))
            for fn in norm_pending:
                fn()

            # ============ Phase 4: O projection + residual (into x_t) ============
            pending = []
            for m in range(PD):
                if m in wobs:
                    wob = wobs.pop(m)
                else:
                    load_wob(m)
                    wob = wobs.pop(m)
                for n in range(2):
                    ps = ps_big.tile([128, 512], FP32, tag="s", name="ops")
                    for k in range(PD):
                        nc.tensor.matmul(
                            ps[:], wob[:, k, :], attnT[k][:, 512 * n:512 * (n + 1)],
                            start=(k == 0), stop=(k == PD - 1))
                    oT = opool.tile([128, 512], FP32, tag="oT", name="oT")
                    nc.scalar.copy(oT[:], ps[:])

                    def emit_o_transposes(oT=oT, m=m, n=n):
                        for j in range(4):
                            t = 4 * n + j
                            tp = ps_av.tile([128, 128], FP32, tag="av", name="tp")
                            nc.tensor.transpose(tp[:], oT[:, 128 * j:128 * (j + 1)],
                                                ident[:])
                            nc.vector.tensor_add(
                                x_t[t][:, 128 * m:128 * (m + 1)], tp[:],
                                x_t[t][:, 128 * m:128 * (m + 1)])
                    pending.append(emit_o_transposes)
                    if len(pending) > 1:
                        pending.pop(0)()
            for fn in pending:
                fn()

            # ================= Phase 5: LN2 -> lnT (reused tags) =================
            ln2T = [res.tile([128, T], BF16, tag=f"lnT{k}", name=f"ln2T{k}")
                    for k in range(PD)]
            layernorm_and_transpose(ln2T)

            # ================= Phase 6: FFN =================
            h1T = [res.tile([128, T], BF16,
                            tag=(f"qk{fm}" if fm < 16 else
                                 f"va{fm - 16}" if fm < 24 else f"at{fm - 24}"),
                            name=f"h1T{fm}")
                   for fm in range(PF)]
            for fm in range(PF):
                wslab = wpool.tile([128, PD, 128], FP32, tag="wslab", name="wslab")
                nc.sync.dma_start(
                    out=wslab[:], in_=w1_r[:, :, 128 * fm:128 * (fm + 1)])
                w1b = w2pool.tile([128, PD, 128], BF16, tag="w2b", name="w1b")
                nc.gpsimd.tensor_copy(w1b[:], wslab[:])
                ps = ps_big.tile([128, T], FP32, tag="s", name="f1")
                for k in range(PD):
                    for n in range(2):
                        nc.tensor.matmul(
                            ps[:, 512 * n:512 * (n + 1)], w1b[:, k, :],
                            ln2T[k][:, 512 * n:512 * (n + 1)],
                            start=(k == 0), stop=(k == PD - 1))
                nc.scalar.activation(h1T[fm][:], ps[:], AF.Gelu)

            pending = []
            for m in range(PD):
                pss = [ps_big.tile([128, 512], FP32, tag="s", name="f2")
                       for _ in range(2)]
                for q in range(4):   # w2 k-range quarters (stream w2 exactly once)
                    wslab = wpool.tile([128, PD, 128], FP32, tag="wslab",
                                       name="wslab")
                    nc.sync.dma_start(
                        out=wslab[:],
                        in_=w2_r[:, 8 * q:8 * (q + 1), 128 * m:128 * (m + 1)])
                    w2b = w2pool.tile([128, PD, 128], BF16, tag="w2b", name="w2b")
                    nc.gpsimd.tensor_copy(w2b[:], wslab[:])
                    for k8 in range(PD):
                        k = 8 * q + k8
                        for n in range(2):
                            nc.tensor.matmul(
                                pss[n][:], w2b[:, k8, :],
                                h1T[k][:, 512 * n:512 * (n + 1)],
                                start=(k == 0), stop=(k == PF - 1))
                for n in range(2):
                    h2 = opool.tile([128, 512], FP32, tag="oT", name="h2")
                    nc.scalar.copy(h2[:], pss[n][:])

                    def emit_out(h2=h2, m=m, n=n):
                        for j in range(4):
                            t = 4 * n + j
                            tp = ps_av.tile([128, 128], FP32, tag="av", name="tp")
                            nc.tensor.transpose(tp[:], h2[:, 128 * j:128 * (j + 1)],
                                                ident[:])
                            ob = obpool.tile([128, 128], FP32, tag="ob", name="ob")
                            nc.vector.tensor_add(ob[:], tp[:],
                                                 x_t[t][:, 128 * m:128 * (m + 1)])
                            nc.sync.dma_start(
                                out=out_r[:, t, 128 * m:128 * (m + 1)], in_=ob[:])
                    pending.append(emit_out)
                    if len(pending) > 1:
                        pending.pop(0)()
            for fn in pending:
                fn()

    nc.finalize()
    return nc


_NC = None


def kernel(**inputs) -> np.ndarray:
    global _NC
    if _NC is None:
        _NC = _build()
    x = np.ascontiguousarray(np.asarray(inputs["x"], dtype=np.float32))
    names = ["w_q", "w_k", "w_v", "w_o", "w1", "w2"]
    ws = {n: np.ascontiguousarray(np.asarray(inputs[n], dtype=np.float32))
          for n in names}
    in_maps = [{"x": x[b], **ws} for b in range(N_CORES)]
    res = run_bass_kernel_spmd(_NC, in_maps, list(range(N_CORES)))
    return np.stack([res.results[b]["out"] for b in range(N_CORES)], axis=0)



# revision 7
# speedup vs baseline: 1.0237x; 1.0237x over previous
"""Transformer encoder layer (LN -> MHA -> residual -> LN -> FFN(erf-GELU) -> residual)
for Trainium2, data-parallel over batch across 8 NeuronCores (one batch element per core).

v3 design vs the fp32r/bf16 baseline (610 us):
- Host pre-casts weights (attention weights to fp8e4, FFN weights + x to bf16) and
  pre-permutes w1/w2 into slab-contiguous layouts (2KB DMA descriptors).
- Q/K projections and O-projection run fp8 DoubleRow (K packed 256 per matmul, 0.5
  cyc/col); AV runs fp8 DoubleRow over token-tile k-pairs. Scores are fp8-normal with
  the two heads of a chunk row-packed into array rows 0:63 / 64:127 (concurrent MMs).
- V and O have natural (token-major) output by making the weights the moving operand
  (no PE transposes); FFN2 output stays feature-major and the final residual
  add + transpose happens on the HOST (out = x2 + ffn2T.T).
- exp is biased by -2 so e = exp(s/8 - 2) fits fp8e4 (softmax is shift-invariant);
  attention is ACT(exp)-bound, everything else hides under it.
- Softmax denominators: raw denom rows bounce through DRAM for the partition
  broadcast into a per-PAIR shared [128,T] tile (head A rows 0:64, head B 64:128);
  ONE batched DVE reciprocal per pair; normalize (mult) is fused into the PSUM
  eviction and flushed at the START of the next pair so the slow ops never gate the
  in-order PE queue (avs slot reuse).
- LN istd uses the cheap [128,1] DVE reciprocal; LN2 stats interleave with the
  O-projection loop so the PE never idles on them.

PSUM (8 banks): tag "big" [128,1024]x2 (scores / QK-chunk accum / V accum / FFN1) = 4
banks; tag "av" [65,512]/[128,512] x4 (avs / LN transposes / O accum / FFN2) = 4.
"""
import numpy as np
import ml_dtypes
from contextlib import ExitStack

import concourse.bass as bass
import concourse.bacc as bacc
import concourse.tile as tile
from concourse import mybir
from concourse.bass_utils import run_bass_kernel_spmd
from concourse.masks import make_identity

N_CORES = 8
T = 1024
D = 1024
H = 16
DH = 64
F = 4096
PT = T // 128
PD = D // 128
PF = F // 128
EPS = 1e-6
EXP_BIAS = -4.0

FP32 = mybir.dt.float32
BF16 = mybir.dt.bfloat16
FP8 = mybir.dt.float8e4
AF = mybir.ActivationFunctionType
DR = mybir.MatmulPerfMode.DoubleRow


def _build():
    nc = bacc.Bacc(None)

    x_d = nc.dram_tensor("x", [T, D], BF16, kind="ExternalInput")
    wq_d = nc.dram_tensor("w_q", [D, D], FP8, kind="ExternalInput")
    wk_d = nc.dram_tensor("w_k", [D, D], FP8, kind="ExternalInput")
    wv_d = nc.dram_tensor("w_v", [D, D], FP8, kind="ExternalInput")
    wo_d = nc.dram_tensor("w_o", [D, D], FP8, kind="ExternalInput")
    # host pre-permuted: w1s [128, PF, PD, 128] = (p, fm, k, mcols)
    w1_d = nc.dram_tensor("w1s", [128, PF, PD, 128], BF16, kind="ExternalInput")
    # host pre-permuted: w2s [128, PD, 4, PD, 128] = (p, m, q, k8, mcols)
    w2_d = nc.dram_tensor("w2s", [128, PD, 4, PD, 128], BF16,
                          kind="ExternalInput")
    x2_d = nc.dram_tensor("x2", [T, D], BF16, kind="ExternalOutput")
    o2_d = nc.dram_tensor("o2", [D, T], BF16, kind="ExternalOutput")

    x_r = x_d.rearrange("(t p) d -> p t d", p=128)
    wq_r = wq_d.rearrange("(k p) m -> p k m", p=128)
    wk_r = wk_d.rearrange("(k p) m -> p k m", p=128)
    wv_r = wv_d.rearrange("(k p) m -> p k m", p=128)
    wo_r = wo_d.rearrange("(k p) m -> p k m", p=128)
    x2_r = x2_d.rearrange("(t p) d -> p t d", p=128)
    o2_r = o2_d.rearrange("(m p) t -> p m t", p=128)

    with tile.TileContext(nc) as tc:
        with ExitStack() as ctx:
            const = ctx.enter_context(tc.tile_pool(name="const", bufs=1))
            res = ctx.enter_context(tc.tile_pool(name="res", bufs=1))
            qkc = ctx.enter_context(tc.tile_pool(name="qkc", bufs=3))
            wsp = ctx.enter_context(tc.tile_pool(name="wsp", bufs=3))
            lnp = ctx.enter_context(tc.tile_pool(name="lnp", bufs=3))
            stp = ctx.enter_context(tc.tile_pool(name="stp", bufs=4))
            dbp = ctx.enter_context(tc.tile_pool(name="dbp", bufs=2))
            dnp = ctx.enter_context(tc.tile_pool(name="dnp", bufs=2))
            obp = ctx.enter_context(tc.tile_pool(name="obp", bufs=4))
            dramp = ctx.enter_context(tc.tile_pool(name="dramp", bufs=2, space="DRAM"))
            psB = ctx.enter_context(tc.tile_pool(name="psB", bufs=2, space="PSUM"))
            psA = ctx.enter_context(tc.tile_pool(name="psA", bufs=4, space="PSUM"))

            ident = const.tile([128, 128], BF16)
            make_identity(nc, ident)
            eps_t = const.tile([128, 1], FP32)
            nc.vector.memset(eps_t[:], EPS)
            ebias_t = const.tile([128, 1], FP32)
            nc.vector.memset(ebias_t[:], EXP_BIAS)

            # ---- resident tensors ----
            x_t = [res.tile([128, D], BF16, tag=f"x{t}", name=f"x{t}")
                   for t in range(PT)]
            lnT = res.tile([128, PD, T], FP8, tag="lnT", name="lnT")
            ln2T = res.tile([128, PD, T], BF16, tag="ln2T", name="ln2T")
            v_pair = [res.tile([128, 2, H, DH + 1], FP8, tag=f"vp{a}", name=f"vp{a}")
                      for a in range(4)]
            attnT = [res.tile([128, 2, T], FP8, tag=f"at{a}", name=f"at{a}")
                     for a in range(4)]
            e_buf = [res.tile([128, 6, T], FP8, tag=f"eb{s}", name=f"eb{s}")
                     for s in range(2)]
            h1T = [res.tile([128, T], BF16, tag=f"h1_{fm}", name=f"h1_{fm}")
                   for fm in range(PF)]
            wq_sb = res.tile([128, PD, D], FP8, tag="wq", name="wq_sb")
            wk_sb = res.tile([128, PD, D], FP8, tag="wk", name="wk_sb")
            wv_sb = res.tile([128, PD, D], FP8, tag="wv", name="wv_sb")
            wo_sb = res.tile([128, PD, D], FP8, tag="wo", name="wo_sb")

            # ---- loads ----
            for t in range(PT):
                nc.sync.dma_start(out=x_t[t][:], in_=x_r[:, t])
            for sb, r in ((wq_sb, wq_r), (wk_sb, wk_r), (wv_sb, wv_r),
                          (wo_sb, wo_r)):
                nc.sync.dma_start(out=sb[:], in_=r[:])
            for a in range(4):
                nc.vector.memset(v_pair[a][:, :, :, DH:DH + 1], 1.0)

            def ln_stats(t):
                stats = stp.tile([128, 2, 6], FP32, tag="bn")
                for i in range(2):
                    nc.vector.bn_stats(out=stats[:, i, :],
                                       in_=x_t[t][:, 512 * i:512 * (i + 1)])
                mv = stp.tile([128, 2], FP32, tag=f"mv{t % 4}")
                nc.vector.bn_aggr(out=mv[:], in_=stats[:])
                istd = stp.tile([128, 1], FP32, tag=f"istd{t % 4}")
                nc.scalar.activation(istd[:], mv[:, 1:2], AF.Sqrt,
                                     bias=eps_t[:], scale=float(D) / (D - 1))
                nc.vector.reciprocal(istd[:], istd[:])
                return mv, istd

            def ln_apply(t, mv, istd, dstT):
                ln_nat = lnp.tile([128, D], BF16, tag="ln_nat")
                nc.vector.tensor_scalar(
                    out=ln_nat[:], in0=x_t[t][:], scalar1=mv[:, 0:1],
                    scalar2=istd[:], op0=mybir.AluOpType.subtract,
                    op1=mybir.AluOpType.mult)
                for g in range(2):
                    tp = psA.tile([128, 512], BF16, tag="av", name="tp")
                    for j in range(4):
                        d8 = 4 * g + j
                        nc.tensor.transpose(
                            tp[:, 128 * j:128 * (j + 1)],
                            ln_nat[:, 128 * d8:128 * (d8 + 1)], ident[:])
                    nc.vector.tensor_copy(
                        dstT[:, 4 * g:4 * (g + 1), 128 * t:128 * (t + 1)],
                        tp[:].rearrange("p (a b) -> p a b", a=4))

            # ================= LN1 =================
            st1 = [ln_stats(t) for t in range(PT)]
            for t in range(PT):
                ln_apply(t, st1[t][0], st1[t][1], lnT)

            # ================= helpers =================
            def qk_chunk(wsb, m, tag):
                big = psB.tile([128, T], FP32, tag="big", name=f"qk{m}")
                for n in range(2):
                    for a in range(4):
                        nc.tensor.matmul(
                            big[:, 512 * n:512 * (n + 1)],
                            wsb[:, 2 * a:2 * a + 2, 128 * m:128 * (m + 1)],
                            lnT[:, 2 * a:2 * a + 2, 512 * n:512 * (n + 1)],
                            start=(a == 0), stop=(a == 3), perf_mode=DR)
                dst = qkc.tile([128, T], FP8, tag=tag, name=tag)
                nc.vector.tensor_copy(dst[:], big[:])
                return dst

            def v_tile(t):
                big = psB.tile([128, T], FP32, tag="big", name=f"v{t}")
                for n in range(2):
                    for k in range(PD):
                        nc.tensor.matmul(
                            big[:, 512 * n:512 * (n + 1)],
                            lnT[:, k, 128 * t:128 * (t + 1)],
                            wv_sb[:, k, 512 * n:512 * (n + 1)],
                            start=(k == 0), stop=(k == PD - 1))
                nc.vector.tensor_copy(
                    v_pair[t // 2][:, t % 2, :, 0:DH],
                    big[:].rearrange("p (h d) -> p h d", d=DH))

            norm_pending = []

            def emit_pair(hp, with_v):
                """Head pair (2hp, 2hp+1): scores (row-packed), exp, AV (fp8-DR,
                trailing), denom bounce, deferred batched normalize."""
                while norm_pending:
                    norm_pending.pop(0)()
                qc = qk_chunk(wq_sb, hp, "qc")
                kc = qk_chunk(wk_sb, hp, "kc")
                avs = {}

                def emit_av(pa):
                    s0 = (2 * pa) % 6
                    for side in range(2):
                        h = 2 * hp + side
                        for n in range(2):
                            if pa == 0:
                                avs[(side, n)] = psA.tile(
                                    [DH + 1, 512], FP32, tag="av", name="avs")
                            nc.tensor.matmul(
                                avs[(side, n)][:],
                                v_pair[pa][:, :, h, :],
                                e_buf[side][:, s0:s0 + 2,
                                            512 * n:512 * (n + 1)],
                                start=(pa == 0), stop=(pa == 3), perf_mode=DR)

                for kt in range(PT):
                    if with_v:
                        v_tile(kt)
                    if kt in (3, 5, 7):
                        emit_av((kt - 3) // 2)
                    ss = [psB.tile([128, T], FP32, tag="big", name="s")
                          for _ in range(2)]
                    for n in range(2):
                        for side in range(2):
                            po = 64 * side
                            nc.tensor.matmul(
                                ss[side][:, 512 * n:512 * (n + 1)],
                                kc[po:po + DH, 128 * kt:128 * (kt + 1)],
                                qc[po:po + DH, 512 * n:512 * (n + 1)],
                                start=True, stop=True)
                    for side in range(2):
                        nc.scalar.activation(e_buf[side][:, kt % 6, :],
                                             ss[side][:], AF.Exp,
                                             scale=0.125, bias=ebias_t[:])
                emit_av(3)

                # denominators: evict row 64 of avs, bounce through DRAM to
                # broadcast; head A -> db rows 0:64, head B -> rows 64:128
                db = dbp.tile([128, T], BF16, tag="db", name="db")
                for side in range(2):
                    po = 64 * side
                    dn = dnp.tile([1, T], BF16, tag="dn", name="dn")
                    for n in range(2):
                        nc.vector.tensor_copy(dn[:, 512 * n:512 * (n + 1)],
                                              avs[(side, n)][DH:DH + 1, :])
                    dd = dramp.tile([1, T], BF16, tag="dd", name="dd")
                    nc.sync.dma_start(out=dd[:], in_=dn[:])
                    src = dd[0:1, :]
                    nc.sync.dma_start(
                        out=db[po:po + DH, :],
                        in_=bass.AP(tensor=src.tensor, offset=src.offset,
                                    ap=[[0, DH]] + list(src.ap[1:])))

                avsnap = dict(avs)

                def normalize(hp=hp, db=db, avs=avsnap):
                    with nc.allow_low_precision(reason="softmax denom"):
                        nc.vector.reciprocal(db[:], db[:])
                    for side in range(2):
                        po = 64 * side
                        h = 2 * hp + side
                        a, j = h // 4, (h // 2) % 2
                        for n in range(2):
                            nc.vector.tensor_mul(
                                attnT[a][po:po + DH, j,
                                         512 * n:512 * (n + 1)],
                                avs[(side, n)][0:DH, :],
                                db[po:po + DH, 512 * n:512 * (n + 1)])
                norm_pending.append(normalize)

            # ================= attention =================
            emit_pair(0, with_v=True)
            for hp in range(1, 8):
                emit_pair(hp, with_v=False)
            for fn in norm_pending:
                fn()
            norm_pending.clear()

            # ====== O projection + residual (natural out) + LN2 stats ======
            st2 = []
            for t in range(PT):
                for n in range(2):
                    ps = psA.tile([128, 512], FP32, tag="av", name="ops")
                    for a in range(4):
                        nc.tensor.matmul(
                            ps[:], attnT[a][:, :, 128 * t:128 * (t + 1)],
                            wo_sb[:, 2 * a:2 * a + 2, 512 * n:512 * (n + 1)],
                            start=(a == 0), stop=(a == 3), perf_mode=DR)
                    nc.vector.tensor_add(x_t[t][:, 512 * n:512 * (n + 1)],
                                         ps[:], x_t[t][:, 512 * n:512 * (n + 1)])
                nc.sync.dma_start(out=x2_r[:, t], in_=x_t[t][:])
                st2.append(ln_stats(t))

            # ================= LN2 apply+transpose =================
            for t in range(PT):
                ln_apply(t, st2[t][0], st2[t][1], ln2T)

            # ================= FFN1 (bf16) =================
            for fm in range(PF):
                w1s = wsp.tile([128, PD, 128], BF16, tag="w1s", name="w1s")
                nc.sync.dma_start(out=w1s[:], in_=w1_d[:, fm])
                big = psB.tile([128, T], FP32, tag="big", name="f1")
                for n in range(2):
                    for k in range(PD):
                        nc.tensor.matmul(
                            big[:, 512 * n:512 * (n + 1)], w1s[:, k, :],
                            ln2T[:, k, 512 * n:512 * (n + 1)],
                            start=(k == 0), stop=(k == PD - 1))
                nc.scalar.activation(h1T[fm][:], big[:], AF.Gelu)

            # ============ FFN2 (bf16, feature-major out; host adds x2) ============
            for m in range(PD):
                pss = [psA.tile([128, 512], FP32, tag="av", name="f2")
                       for _ in range(2)]
                for q in range(4):
                    w2s = wsp.tile([128, PD, 128], BF16, tag="w2s", name="w2s")
                    nc.sync.dma_start(out=w2s[:], in_=w2_d[:, m, q])
                    for k8 in range(PD):
                        k = 8 * q + k8
                        for n in range(2):
                            nc.tensor.matmul(
                                pss[n][:], w2s[:, k8, :],
                                h1T[k][:, 512 * n:512 * (n + 1)],
                                start=(k == 0), stop=(k == PF - 1))
                for n in range(2):
                    ob = obp.tile([128, 512], BF16, tag="ob", name="ob")
                    nc.vector.tensor_copy(ob[:], pss[n][:])
                    nc.sync.dma_start(
                        out=o2_r[:, m, 512 * n:512 * (n + 1)], in_=ob[:])

    nc.finalize()
    return nc


_NC = None


def prepare_in_maps(inputs):
    f8 = ml_dtypes.float8_e4m3
    bf = ml_dtypes.bfloat16
    x = np.asarray(inputs["x"], dtype=np.float32).astype(bf)
    ws = {n: np.ascontiguousarray(
            np.asarray(inputs[n], dtype=np.float32)).astype(f8)
          for n in ("w_q", "w_k", "w_v", "w_o")}
    # w1 [D, F] -> [128, PF, PD, 128]: (p, fm, k, mcols), p = d % 128, k = d // 128
    w1 = np.asarray(inputs["w1"], dtype=np.float32).astype(bf)
    ws["w1s"] = np.ascontiguousarray(
        w1.reshape(PD, 128, PF, 128).transpose(1, 2, 0, 3))
    # w2 [F, D] -> [128, PD, 4, PD, 128]: (p, m, q, k8, mcols), p = f % 128,
    # q*8+k8 = f // 128
    w2 = np.asarray(inputs["w2"], dtype=np.float32).astype(bf)
    ws["w2s"] = np.ascontiguousarray(
        w2.reshape(4, PD, 128, PD, 128).transpose(2, 3, 0, 1, 4))
    return [{"x": np.ascontiguousarray(x[b]), **ws} for b in range(N_CORES)]


def combine_outputs(res):
    out = np.empty((N_CORES, T, D), dtype=np.float32)
    for b in range(N_CORES):
        r = res.results[b]
        out[b] = (r["x2"].astype(np.float32)
                  + r["o2"].astype(np.float32).T)
    return out


def kernel(**inputs) -> np.ndarray:
    global _NC
    if _NC is None:
        _NC = _build()
    in_maps = prepare_in_maps(inputs)
    res = run_bass_kernel_spmd(_NC, in_maps, list(range(N_CORES)))
    return combine_outputs(res)


# revision 11
# speedup vs baseline: 1.0246x; 1.0009x over previous
"""Transformer encoder layer (LN -> MHA -> residual -> LN -> FFN(erf-GELU) -> residual)
for Trainium2, data-parallel over batch across 8 NeuronCores (one batch element per core).

v3 design vs the fp32r/bf16 baseline (610 us):
- Host pre-casts weights (attention weights to fp8e4, FFN weights + x to bf16) and
  pre-permutes w1/w2 into slab-contiguous layouts (2KB DMA descriptors).
- Q/K projections and O-projection run fp8 DoubleRow (K packed 256 per matmul, 0.5
  cyc/col); AV runs fp8 DoubleRow over token-tile k-pairs. Scores are fp8-normal with
  the two heads of a chunk row-packed into array rows 0:63 / 64:127 (concurrent MMs).
- V and O have natural (token-major) output by making the weights the moving operand
  (no PE transposes); FFN2 output stays feature-major and the final residual
  add + transpose happens on the HOST (out = x2 + ffn2T.T).
- exp is biased by -2 so e = exp(s/8 - 2) fits fp8e4 (softmax is shift-invariant);
  attention is ACT(exp)-bound, everything else hides under it.
- Softmax denominators: raw denom rows bounce through DRAM for the partition
  broadcast into a per-PAIR shared [128,T] tile (head A rows 0:64, head B 64:128);
  ONE batched DVE reciprocal per pair; normalize (mult) is fused into the PSUM
  eviction and flushed at the START of the next pair so the slow ops never gate the
  in-order PE queue (avs slot reuse).
- LN istd uses the cheap [128,1] DVE reciprocal; LN2 stats interleave with the
  O-projection loop so the PE never idles on them.

PSUM (8 banks): tag "big" [128,1024]x2 (scores / QK-chunk accum / V accum / FFN1) = 4
banks; tag "av" [65,512]/[128,512] x4 (avs / LN transposes / O accum / FFN2) = 4.
"""
import numpy as np
import ml_dtypes
from contextlib import ExitStack

import concourse.bass as bass
import concourse.bacc as bacc
import concourse.tile as tile
from concourse import mybir
from concourse.bass_utils import run_bass_kernel_spmd
from concourse.masks import make_identity

N_CORES = 8
T = 1024
D = 1024
H = 16
DH = 64
F = 4096
PT = T // 128
PD = D // 128
PF = F // 128
EPS = 1e-6
EXP_BIAS = -4.0

FP32 = mybir.dt.float32
BF16 = mybir.dt.bfloat16
FP8 = mybir.dt.float8e4
AF = mybir.ActivationFunctionType
DR = mybir.MatmulPerfMode.DoubleRow


def _build():
    nc = bacc.Bacc(None)

    x_d = nc.dram_tensor("x", [T, D], BF16, kind="ExternalInput")
    wq_d = nc.dram_tensor("w_q", [D, D], FP8, kind="ExternalInput")
    wk_d = nc.dram_tensor("w_k", [D, D], FP8, kind="ExternalInput")
    wv_d = nc.dram_tensor("w_v", [D, D], FP8, kind="ExternalInput")
    wo_d = nc.dram_tensor("w_o", [D, D], FP8, kind="ExternalInput")
    # host pre-permuted: w1s [128, PF, PD, 128] = (p, fm, k, mcols)
    w1_d = nc.dram_tensor("w1s", [128, PF, PD, 128], BF16, kind="ExternalInput")
    # host pre-permuted: w2s [128, PD, 4, PD, 128] = (p, m, q, k8, mcols)
    w2_d = nc.dram_tensor("w2s", [128, PD, 4, PD, 128], BF16,
                          kind="ExternalInput")
    x2_d = nc.dram_tensor("x2", [T, D], BF16, kind="ExternalOutput")
    o2_d = nc.dram_tensor("o2", [D, T], BF16, kind="ExternalOutput")

    x_r = x_d.rearrange("(t p) d -> p t d", p=128)
    wq_r = wq_d.rearrange("(k p) m -> p k m", p=128)
    wk_r = wk_d.rearrange("(k p) m -> p k m", p=128)
    wv_r = wv_d.rearrange("(k p) m -> p k m", p=128)
    wo_r = wo_d.rearrange("(k p) m -> p k m", p=128)
    x2_r = x2_d.rearrange("(t p) d -> p t d", p=128)
    o2_r = o2_d.rearrange("(m p) t -> p m t", p=128)

    with tile.TileContext(nc) as tc:
        with ExitStack() as ctx:
            const = ctx.enter_context(tc.tile_pool(name="const", bufs=1))
            res = ctx.enter_context(tc.tile_pool(name="res", bufs=1))
            qkc = ctx.enter_context(tc.tile_pool(name="qkc", bufs=3))
            wsp = ctx.enter_context(tc.tile_pool(name="wsp", bufs=3))
            lnp = ctx.enter_context(tc.tile_pool(name="lnp", bufs=3))
            stp = ctx.enter_context(tc.tile_pool(name="stp", bufs=4))
            dbp = ctx.enter_context(tc.tile_pool(name="dbp", bufs=2))
            dnp = ctx.enter_context(tc.tile_pool(name="dnp", bufs=2))
            obp = ctx.enter_context(tc.tile_pool(name="obp", bufs=4))
            dramp = ctx.enter_context(tc.tile_pool(name="dramp", bufs=2, space="DRAM"))
            psB = ctx.enter_context(tc.tile_pool(name="psB", bufs=2, space="PSUM"))
            psA = ctx.enter_context(tc.tile_pool(name="psA", bufs=4, space="PSUM"))

            ident = const.tile([128, 128], BF16)
            make_identity(nc, ident)
            eps_t = const.tile([128, 1], FP32)
            nc.vector.memset(eps_t[:], EPS)
            ebias_t = const.tile([128, 1], FP32)
            nc.vector.memset(ebias_t[:], EXP_BIAS)

            # ---- resident tensors ----
            x_t = [res.tile([128, D], BF16, tag=f"x{t}", name=f"x{t}")
                   for t in range(PT)]
            lnT = res.tile([128, PD, T], FP8, tag="lnT", name="lnT")
            ln2T = res.tile([128, PD, T], BF16, tag="ln2T", name="ln2T")
            v_pair = [res.tile([128, 2, H, DH + 1], FP8, tag=f"vp{a}", name=f"vp{a}")
                      for a in range(4)]
            attnT = [res.tile([128, 2, T], FP8, tag=f"at{a}", name=f"at{a}")
                     for a in range(4)]
            e_buf = [res.tile([128, PT, T], FP8, tag=f"eb{s}", name=f"eb{s}")
                     for s in range(2)]
            h1T = [res.tile([128, T], BF16, tag=f"h1_{fm}", name=f"h1_{fm}")
                   for fm in range(PF)]
            wq_sb = res.tile([128, PD, D], FP8, tag="wq", name="wq_sb")
            wk_sb = res.tile([128, PD, D], FP8, tag="wk", name="wk_sb")
            wv_sb = res.tile([128, PD, D], FP8, tag="wv", name="wv_sb")
            wo_sb = res.tile([128, PD, D], FP8, tag="wo", name="wo_sb")

            # ---- loads ----
            for t in range(PT):
                nc.sync.dma_start(out=x_t[t][:], in_=x_r[:, t])
            for sb, r in ((wq_sb, wq_r), (wk_sb, wk_r), (wv_sb, wv_r),
                          (wo_sb, wo_r)):
                nc.sync.dma_start(out=sb[:], in_=r[:])
            for a in range(4):
                nc.vector.memset(v_pair[a][:, :, :, DH:DH + 1], 1.0)

            def ln_stats(t):
                stats = stp.tile([128, 2, 6], FP32, tag="bn")
                for i in range(2):
                    nc.vector.bn_stats(out=stats[:, i, :],
                                       in_=x_t[t][:, 512 * i:512 * (i + 1)])
                mv = stp.tile([128, 2], FP32, tag=f"mv{t % 4}")
                nc.vector.bn_aggr(out=mv[:], in_=stats[:])
                istd = stp.tile([128, 1], FP32, tag=f"istd{t % 4}")
                nc.scalar.activation(istd[:], mv[:, 1:2], AF.Sqrt,
                                     bias=eps_t[:], scale=float(D) / (D - 1))
                nc.vector.reciprocal(istd[:], istd[:])
                return mv, istd

            def ln_apply(t, mv, istd, dstT):
                ln_nat = lnp.tile([128, D], BF16, tag="ln_nat")
                nc.vector.tensor_scalar(
                    out=ln_nat[:], in0=x_t[t][:], scalar1=mv[:, 0:1],
                    scalar2=istd[:], op0=mybir.AluOpType.subtract,
                    op1=mybir.AluOpType.mult)
                for g in range(2):
                    tp = psA.tile([128, 512], BF16, tag="av", name="tp")
                    for j in range(4):
                        d8 = 4 * g + j
                        nc.tensor.transpose(
                            tp[:, 128 * j:128 * (j + 1)],
                            ln_nat[:, 128 * d8:128 * (d8 + 1)], ident[:])
                    nc.vector.tensor_copy(
                        dstT[:, 4 * g:4 * (g + 1), 128 * t:128 * (t + 1)],
                        tp[:].rearrange("p (a b) -> p a b", a=4))

            # ================= LN1 =================
            for t in range(PT):
                mv, istd = ln_stats(t)
                ln_apply(t, mv, istd, lnT)

            # ================= helpers =================
            def qk_chunk(wsb, m, tag):
                big = psB.tile([128, T], FP32, tag="big", name=f"qk{m}")
                for n in range(2):
                    for a in range(4):
                        nc.tensor.matmul(
                            big[:, 512 * n:512 * (n + 1)],
                            wsb[:, 2 * a:2 * a + 2, 128 * m:128 * (m + 1)],
                            lnT[:, 2 * a:2 * a + 2, 512 * n:512 * (n + 1)],
                            start=(a == 0), stop=(a == 3), perf_mode=DR)
                dst = qkc.tile([128, T], FP8, tag=tag, name=tag)
                nc.vector.tensor_copy(dst[:], big[:])
                return dst

            def v_tile(t):
                big = psB.tile([128, T], FP32, tag="big", name=f"v{t}")
                for n in range(2):
                    for k in range(PD):
                        nc.tensor.matmul(
                            big[:, 512 * n:512 * (n + 1)],
                            lnT[:, k, 128 * t:128 * (t + 1)],
                            wv_sb[:, k, 512 * n:512 * (n + 1)],
                            start=(k == 0), stop=(k == PD - 1))
                nc.vector.tensor_copy(
                    v_pair[t // 2][:, t % 2, :, 0:DH],
                    big[:].rearrange("p (h d) -> p h d", d=DH))

            # ---------------- attention machinery ----------------
            # Software-pipelined: pair p's AV(pa=2,3) and pair p+1's Q/K
            # chunks run as a dense PE burst at the START of pair p+1 (keeps
            # HAM unthrottled and ACT continuous); AV(pa=0,1) run mid-pair
            # once their exps land. Denominator bounce + normalize trail by
            # one pair.
            pair_state = {}  # hp -> dict(avs=..., qc=..., kc=...)

            def emit_scores_kt(hp, kt, st):
                qc, kc = st["qc"], st["kc"]
                ss = [psB.tile([128, T], FP32, tag="big", name="s")
                      for _ in range(2)]
                for n in range(2):
                    for side in range(2):
                        po = 64 * side
                        nc.tensor.matmul(
                            ss[side][:, 512 * n:512 * (n + 1)],
                            kc[po:po + DH, 128 * kt:128 * (kt + 1)],
                            qc[po:po + DH, 512 * n:512 * (n + 1)],
                            start=True, stop=True)
                for side in range(2):
                    nc.scalar.activation(e_buf[side][:, kt, :],
                                         ss[side][:], AF.Exp,
                                         scale=0.125, bias=ebias_t[:])

            def emit_av(hp, pa, st):
                for side in range(2):
                    h = 2 * hp + side
                    for n in range(2):
                        if pa == 0:
                            st["avs"][(side, n)] = psA.tile(
                                [DH + 1, 512], FP32, tag="av", name="avs")
                        nc.tensor.matmul(
                            st["avs"][(side, n)][:],
                            v_pair[pa][:, :, h, :],
                            e_buf[side][:, 2 * pa:2 * pa + 2,
                                        512 * n:512 * (n + 1)],
                            start=(pa == 0), stop=(pa == 3), perf_mode=DR)

            def emit_denom(hp, st):
                db = dbp.tile([128, T], BF16, tag="db", name="db")
                st["db"] = db
                for side in range(2):
                    po = 64 * side
                    dn = dnp.tile([1, T], BF16, tag="dn", name="dn")
                    for n in range(2):
                        nc.vector.tensor_copy(
                            dn[:, 512 * n:512 * (n + 1)],
                            st["avs"][(side, n)][DH:DH + 1, :])
                    dd = dramp.tile([1, T], BF16, tag="dd", name="dd")
                    nc.sync.dma_start(out=dd[:], in_=dn[:])
                    src = dd[0:1, :]
                    nc.sync.dma_start(
                        out=db[po:po + DH, :],
                        in_=bass.AP(tensor=src.tensor, offset=src.offset,
                                    ap=[[0, DH]] + list(src.ap[1:])))

            def emit_normalize(hp, st):
                with nc.allow_low_precision(reason="softmax denom"):
                    nc.vector.reciprocal(st["db"][:], st["db"][:])
                for side in range(2):
                    po = 64 * side
                    h = 2 * hp + side
                    a, j = h // 4, (h // 2) % 2
                    for n in range(2):
                        nc.vector.tensor_mul(
                            attnT[a][po:po + DH, j, 512 * n:512 * (n + 1)],
                            st["avs"][(side, n)][0:DH, :],
                            st["db"][po:po + DH, 512 * n:512 * (n + 1)])

            # ================= attention =================
            pair_state[0] = {"qc": qk_chunk(wq_sb, 0, "qc"),
                             "kc": qk_chunk(wk_sb, 0, "kc"), "avs": {}}
            for hp in range(8):
                st = pair_state[hp]
                for kt in range(2):
                    if hp == 0:
                        v_tile(kt)
                    emit_scores_kt(hp, kt, st)
                if hp >= 1:
                    prev = pair_state[hp - 1]
                    emit_av(hp - 1, 2, prev)
                    emit_av(hp - 1, 3, prev)
                    emit_denom(hp - 1, prev)
                if hp < 7:
                    pair_state[hp + 1] = {
                        "qc": qk_chunk(wq_sb, hp + 1, "qc"),
                        "kc": qk_chunk(wk_sb, hp + 1, "kc"), "avs": {}}
                for kt in range(2, PT):
                    if hp == 0:
                        v_tile(kt)
                    emit_scores_kt(hp, kt, st)
                    if kt == 4 and hp >= 1:
                        emit_normalize(hp - 1, pair_state[hp - 1])
                    if kt == 5:
                        emit_av(hp, 0, st)
                    if kt == 7:
                        emit_av(hp, 1, st)
            # tail: finish pair 7
            st = pair_state[7]
            emit_av(7, 2, st)
            emit_av(7, 3, st)
            emit_denom(7, st)
            emit_normalize(7, st)

            # ====== O projection + residual (natural out) + fused LN2 ======
            for t in range(PT):
                for n in range(2):
                    ps = psA.tile([128, 512], FP32, tag="av", name="ops")
                    for a in range(4):
                        nc.tensor.matmul(
                            ps[:], attnT[a][:, :, 128 * t:128 * (t + 1)],
                            wo_sb[:, 2 * a:2 * a + 2, 512 * n:512 * (n + 1)],
                            start=(a == 0), stop=(a == 3), perf_mode=DR)
                    nc.vector.tensor_add(x_t[t][:, 512 * n:512 * (n + 1)],
                                         ps[:], x_t[t][:, 512 * n:512 * (n + 1)])
                nc.sync.dma_start(out=x2_r[:, t], in_=x_t[t][:])
                mv, istd = ln_stats(t)
                ln_apply(t, mv, istd, ln2T)

            # ================= FFN1 (bf16) =================
            for fm in range(PF):
                w1s = wsp.tile([128, PD, 128], BF16, tag="w1s", name="w1s")
                nc.sync.dma_start(out=w1s[:], in_=w1_d[:, fm])
                big = psB.tile([128, T], FP32, tag="big", name="f1")
                for n in range(2):
                    for k in range(PD):
                        nc.tensor.matmul(
                            big[:, 512 * n:512 * (n + 1)], w1s[:, k, :],
                            ln2T[:, k, 512 * n:512 * (n + 1)],
                            start=(k == 0), stop=(k == PD - 1))
                nc.scalar.activation(h1T[fm][:], big[:], AF.Gelu)

            # ============ FFN2 (bf16, feature-major out; host adds x2) ============
            for m in range(PD):
                pss = [psA.tile([128, 512], FP32, tag="av", name="f2")
                       for _ in range(2)]
                for q in range(4):
                    w2s = wsp.tile([128, PD, 128], BF16, tag="w2s", name="w2s")
                    nc.sync.dma_start(out=w2s[:], in_=w2_d[:, m, q])
                    for k8 in range(PD):
                        k = 8 * q + k8
                        for n in range(2):
                            nc.tensor.matmul(
                                pss[n][:], w2s[:, k8, :],
                                h1T[k][:, 512 * n:512 * (n + 1)],
                                start=(k == 0), stop=(k == PF - 1))
                for n in range(2):
                    ob = obp.tile([128, 512], BF16, tag="ob", name="ob")
                    nc.vector.tensor_copy(ob[:], pss[n][:])
                    nc.sync.dma_start(
                        out=o2_r[:, m, 512 * n:512 * (n + 1)], in_=ob[:])

    nc.finalize()
    return nc


_NC = None


def prepare_in_maps(inputs):
    f8 = ml_dtypes.float8_e4m3
    bf = ml_dtypes.bfloat16
    x = np.asarray(inputs["x"], dtype=np.float32).astype(bf)
    ws = {n: np.ascontiguousarray(
            np.asarray(inputs[n], dtype=np.float32)).astype(f8)
          for n in ("w_q", "w_k", "w_v", "w_o")}
    # w1 [D, F] -> [128, PF, PD, 128]: (p, fm, k, mcols), p = d % 128, k = d // 128
    w1 = np.asarray(inputs["w1"], dtype=np.float32).astype(bf)
    ws["w1s"] = np.ascontiguousarray(
        w1.reshape(PD, 128, PF, 128).transpose(1, 2, 0, 3))
    # w2 [F, D] -> [128, PD, 4, PD, 128]: (p, m, q, k8, mcols), p = f % 128,
    # q*8+k8 = f // 128
    w2 = np.asarray(inputs["w2"], dtype=np.float32).astype(bf)
    ws["w2s"] = np.ascontiguousarray(
        w2.reshape(4, PD, 128, PD, 128).transpose(2, 3, 0, 1, 4))
    return [{"x": np.ascontiguousarray(x[b]), **ws} for b in range(N_CORES)]


def combine_outputs(res):
    out = np.empty((N_CORES, T, D), dtype=np.float32)
    for b in range(N_CORES):
        r = res.results[b]
        out[b] = (r["x2"].astype(np.float32)
                  + r["o2"].astype(np.float32).T)
    return out


def kernel(**inputs) -> np.ndarray:
    global _NC
    if _NC is None:
        _NC = _build()
    in_maps = prepare_in_maps(inputs)
    res = run_bass_kernel_spmd(_NC, in_maps, list(range(N_CORES)))
    return combine_outputs(res)
